# revision 1
# baseline (speedup 1.0000x reference)
"""Peephole-LSTM cell fused kernel for 8 Trainium2 NeuronCores.

Math (per reference):
    pre = X_t @ W + c_prev @ U + b          # W/U/b are the 4 gates concat'd
    f, i, o, c_hat = split(pre);  f,i,o = sigmoid;  c_hat = tanh
    c_t = f * c_prev + i * c_hat
    h_t = o * tanh(c_t)

Sharding: data-parallel over the batch dim (16384 -> 8 x 2048), weights
replicated, no cross-device communication.

Per-core device plan (B_loc=2048, D=512, 4H=2048):
  - Host pre-transposes X and c_prev and casts to fp16, so the device reads
    X^T / C^T [k, b] with plain contiguous DMAs; the tensor engine does
    nothing but the 512 N=512 matmuls (fp16 runs at full PE rate; fp32
    matmul is 4x slower, fp8 too inaccurate; rel err ~1e-3).
  - Startup is HBM-bound (8 MB of operands at ~360 GB/s) and one DMA queue
    only sustains ~110 GB/s, so operands are split into 32 quarter-MB
    pieces arranged in 4 concurrent serial chains (lanes, two transfers in
    flight each) whose arrival order matches consumption: X^T/W (k-major)
    first, then C^T/U, then the batch-halves needed late.
  - Phase A runs X@W for the first N_STAGED tiles in k-outer order over
    PAIRS of batch tiles (2 tiles x 4 gates = all 8 PSUM banks), staging
    partials to SBUF fp16, so the PE consumes each arriving k-piece wave
    at the rate it lands.  Remaining tiles run fused 8-matmul PSUM
    accumulation.
  - PSUM is managed as single-bank [128,512] tiles, one per gate, filled
    (c_hat, i, f, o) so the deep elementwise chain overlaps the matmuls.
  - ScalarE does sigmoid/tanh straight from PSUM; the elementwise chain is
    split across VectorE and GpSimd (all-fast-engine for the final tile's
    exposed tail); c_prev is re-loaded fp32-natural for full precision in
    f * c_prev.
"""

import sys

if "/opt/trn_rl_repo" not in sys.path:
    sys.path.insert(0, "/opt/trn_rl_repo")

import numpy as np

import concourse.bacc as bacc
import concourse.mybir as mybir
import concourse.tile as tile
from concourse import bass_utils

N_CORES = 8
B, D, H = 16384, 512, 512
BL = B // N_CORES          # 2048 rows per core
G4 = 4 * H                 # 2048, the concatenated gate dim
KT = D // 128              # 4 k-tiles
BT = BL // 128             # 16 batch tiles per core
WARMUP_MMS = 8             # junk matmuls to lift the HAM clock gate
N_STAGED = 4               # batch tiles that run X@W-only while C/U load
BANK_ORDER = (3, 1, 0, 2)  # c_hat, i, f, o — deep chain first
LAST_ORDER = (3, 0, 1, 2)  # final tile: f early so c_t closes pre-end

_cached = {}


def _build(has_bias: bool):
    key = has_bias
    if key in _cached:
        return _cached[key]

    f32 = mybir.dt.float32
    f16 = mybir.dt.float16
    AF = mybir.ActivationFunctionType
    Alu = mybir.AluOpType

    nc = bacc.Bacc("TRN2", target_bir_lowering=False, debug=False,
                   enable_asserts=False, enable_partition_id=False)
    xT = nc.dram_tensor("xT_f16", [D, BL], f16, kind="ExternalInput")
    cT = nc.dram_tensor("cT_f16", [D, BL], f16, kind="ExternalInput")
    c_f32 = nc.dram_tensor("c_f32", [BL, D], f32, kind="ExternalInput")
    w_f16 = nc.dram_tensor("w_f16", [D, G4], f16, kind="ExternalInput")
    u_f16 = nc.dram_tensor("u_f16", [D, G4], f16, kind="ExternalInput")
    if has_bias:
        bias_bc = nc.dram_tensor("bias_bc", [128, G4], f32, kind="ExternalInput")
    h_out = nc.dram_tensor("h_out", [BL, H], f32, kind="ExternalOutput")

    with tile.TileContext(nc) as tc:
        with (
            tc.tile_pool(name="const", bufs=1) as const,
            tc.tile_pool(name="px", bufs=1) as px_p,
            tc.tile_pool(name="psum", bufs=8, space="PSUM") as psum,
            tc.tile_pool(name="cnat", bufs=4) as cnat,
            tc.tile_pool(name="gates", bufs=8) as gate_p,
            tc.tile_pool(name="tmp1", bufs=3) as tmp1_p,
            tc.tile_pool(name="tmp2", bufs=3) as tmp2_p,
            tc.tile_pool(name="hsb", bufs=3) as h_p,
        ):
            # PE warm-up: the HAM clock gate holds the PE at 1.2 GHz
            # until it sees ~3.4 us of sustained activity.  The PE would
            # otherwise idle from engine-boot (~3.5 us) until the first
            # operands land (~11 us) and then run its first ~12 real
            # matmuls at half clock.  Burn that window on junk matmuls
            # over a zeroed tile so real matmuls start warm.
            junk = const.tile([128, 512], f16, tag="junk", name="junk")
            nc.vector.memset(junk[:], 0.0)
            # Target the real pair-0 PSUM banks (each real accumulation
            # group begins start=True, which resets them) so bacc's DCE
            # keeps these otherwise-dead matmuls.
            ps4_pair0 = {
                (bt, jc): psum.tile(
                    [128, 512], f32, tag="ps", name=f"psA{bt}_{jc}"
                )
                for bt in (0, 1) for jc in range(4)
            }
            for i in range(WARMUP_MMS):
                tgt = ps4_pair0[((i // 4) % 2, i % 4)]
                nc.tensor.matmul(
                    tgt[:], junk[:, 0:128], junk[:], start=True, stop=True
                )

            # 32 operand pieces of [128, 1024] fp16 (256 KiB each):
            # xt/ct split by batch half (a = batch tiles 0-7), w/u by
            # gate pair (a = banks 0-1, b = banks 2-3).
            def piece(name):
                return const.tile([128, 1024], f16, tag=name, name=name)

            XTh = [[piece(f"xt{k}h{q}") for q in range(2)] for k in range(KT)]
            CTh = [[piece(f"ct{k}h{q}") for q in range(2)] for k in range(KT)]
            Wh = [[piece(f"w{k}h{j}") for j in range(2)] for k in range(KT)]
            Uh = [[piece(f"u{k}h{j}") for j in range(2)] for k in range(KT)]

            def dram(t, k, half):
                return t.ap()[k * 128:(k + 1) * 128,
                              half * 1024:(half + 1) * 1024]

            # Priority order = consumption order.
            prio = []
            # Block 1: X^T(a)/W, k-major, for phase A and fused tiles 4-7.
            for k in range(KT):
                prio.append((XTh[k][0], xT, k, 0))
                prio.append((Wh[k][1], w_f16, k, 1))
                prio.append((Wh[k][0], w_f16, k, 0))
            # Block 2: C^T(a)/U for staged-B and fused tiles 4-7.
            for k in range(KT):
                prio.append((CTh[k][0], cT, k, 0))
                prio.append((Uh[k][1], u_f16, k, 1))
                prio.append((Uh[k][0], u_f16, k, 0))
            block2_end = prio[-1][0]
            # Block 3: batch b-halves, needed from batch tile 8 on.
            for k in range(KT):
                prio.append((XTh[k][1], xT, k, 1))
            for k in range(KT):
                prio.append((CTh[k][1], cT, k, 1))

            # 4 lanes (strided), all issued on sync (a gated DMA blocks
            # its issuing engine; scalar must stay free for ACT).  Each
            # piece gates on the piece TWO back in its lane: 2 transfers
            # in flight per lane hide the per-piece semaphore/issue
            # latency while keeping near-priority arrival order.
            n_lanes = 4
            lanes = [prio[i::n_lanes] for i in range(n_lanes)]
            for wave in range(len(lanes[0])):
                for li, lane in enumerate(lanes):
                    if wave >= len(lane):
                        continue
                    dst, t, k, half = lane[wave]
                    if wave > 1:
                        prev = lane[wave - 2][0]
                        nc.vector.tensor_copy(dst[0:1, 0:1], prev[0:1, 0:1])
                        eng = nc.sync
                    else:
                        # Ungated first waves: split issue across both
                        # HWDGE engines (they won't block scalar's ACT
                        # stream since they carry no semaphore gate).
                        eng = nc.scalar if li % 2 else nc.sync
                    eng.dma_start(out=dst[:], in_=dram(t, k, half))
            if has_bias:
                bias_sb = const.tile([128, G4], f32, tag="bias")
                nc.sync.dma_start(out=bias_sb[:], in_=bias_bc.ap())

            def lhsq(T, k, bt):
                q, r = divmod(bt, 8)
                return T[k][q][:, r * 128:(r + 1) * 128]

            def rhsj(T, k, jc):
                q, r = divmod(jc, 2)
                return T[k][q][:, r * 512:(r + 1) * 512]

            # Phase A: X@W only, k-outer over pairs of batch tiles
            # (2 tiles x 4 gates = 8 PSUM banks), staged to SBUF fp16.
            PX = []
            for bt in range(N_STAGED):
                PX.append(px_p.tile([128, G4], f16, tag=f"px{bt}", name=f"px{bt}"))
            for pr in range(N_STAGED // 2):
                bts = (2 * pr, 2 * pr + 1)
                if pr == 0:
                    ps4 = ps4_pair0
                else:
                    ps4 = {
                        (bt, jc): psum.tile(
                            [128, 512], f32, tag="ps", name=f"psA{bt}_{jc}"
                        )
                        for bt in bts for jc in range(4)
                    }
                for k in range(KT):
                    # Banks 3,2 read the W b-half piece, banks 1,0 the
                    # a-half; consume each arriving piece fully (across
                    # both tiles of the pair) before needing the next.
                    for jc in (3, 2, 1, 0):
                        for bt in bts:
                            nc.tensor.matmul(
                                ps4[bt, jc][:], lhsq(XTh, k, bt), rhsj(Wh, k, jc),
                                start=(k == 0), stop=(k == KT - 1),
                            )
                for bt in bts:
                    for jc in range(4):
                        nc.scalar.activation(
                            PX[bt][:, jc * 512:(jc + 1) * 512],
                            ps4[bt, jc][:], AF.Copy,
                        )

            first_cn = True
            # Phase B: per tile, fill the four gate banks and run the
            # eviction/elementwise chain bank-by-bank.
            for bt in range(BT):
                bsl = slice(bt * 128, (bt + 1) * 128)
                last = bt == BT - 1
                cn = cnat.tile([128, H], f32, tag="cn", name=f"cn{bt}")
                if first_cn:
                    # Keep c_prev-natural loads behind the operand lanes.
                    nc.vector.tensor_copy(cn[0:1, 0:1], block2_end[0:1, 0:1])
                    first_cn = False
                nc.sync.dma_start(out=cn[:], in_=c_f32.ap()[bsl, :])

                staged = bt < N_STAGED
                order = LAST_ORDER if last else BANK_ORDER
                gates = {}
                for jc in order:
                    jsl = slice(jc * 512, (jc + 1) * 512)
                    ps = psum.tile([128, 512], f32, tag="ps", name=f"psB{bt}_{jc}")
                    if staged:
                        for k in range(KT):
                            nc.tensor.matmul(
                                ps[:], lhsq(CTh, k, bt), rhsj(Uh, k, jc),
                                start=(k == 0), stop=(k == KT - 1),
                            )
                        nc.vector.tensor_tensor(
                            ps[:], ps[:], PX[bt][:, jsl], Alu.add
                        )
                    else:
                        for k in range(KT):
                            nc.tensor.matmul(
                                ps[:], lhsq(XTh, k, bt), rhsj(Wh, k, jc),
                                start=(k == 0), stop=False,
                            )
                        for k in range(KT):
                            nc.tensor.matmul(
                                ps[:], lhsq(CTh, k, bt), rhsj(Uh, k, jc),
                                start=False, stop=(k == KT - 1),
                            )
                    if has_bias:
                        nc.vector.tensor_tensor(
                            ps[:], ps[:], bias_sb[:, jsl], Alu.add
                        )
                    g = gate_p.tile([128, 512], f32, tag="g", name=f"g{bt}_{jc}")
                    if last and jc == 2:
                        for hs in range(2):
                            csl = slice(hs * 256, (hs + 1) * 256)
                            nc.scalar.activation(
                                g[:, csl], ps[:, csl], AF.Sigmoid
                            )
                    else:
                        nc.scalar.activation(
                            g[:], ps[:], AF.Tanh if jc == 3 else AF.Sigmoid
                        )
                    gates[jc] = g
                    # Chain steps as their inputs become ready.
                    if jc == 0:
                        t1 = tmp1_p.tile([128, H], f32, tag="t1", name=f"t1_{bt}")
                        if last:
                            nc.vector.tensor_tensor(
                                t1[:], gates[0][:], cn[:], Alu.mult
                            )
                        else:
                            nc.gpsimd.tensor_tensor(
                                t1[:], gates[0][:], cn[:], Alu.mult
                            )
                    elif jc == 1:
                        t2 = tmp2_p.tile([128, H], f32, tag="t2", name=f"t2_{bt}")
                        nc.vector.tensor_tensor(
                            t2[:], gates[1][:], gates[3][:], Alu.mult
                        )
                        if last:
                            nc.vector.tensor_tensor(t1[:], t1[:], t2[:], Alu.add)
                            tct = tmp2_p.tile(
                                [128, H], f32, tag="tct", name=f"tct{bt}"
                            )
                            nc.scalar.activation(tct[:], t1[:], AF.Tanh)
                    elif jc == 2:
                        if not last:
                            nc.vector.tensor_tensor(t1[:], t1[:], t2[:], Alu.add)
                            tct = tmp2_p.tile(
                                [128, H], f32, tag="tct", name=f"tct{bt}"
                            )
                            nc.scalar.activation(tct[:], t1[:], AF.Tanh)
                        hsb = h_p.tile([128, H], f32, tag="h", name=f"h{bt}")
                        if last:
                            # Halve the exposed tail: evict/multiply/store
                            # the o-gate in 256-col chunks.
                            for hs in range(2):
                                csl = slice(hs * 256, (hs + 1) * 256)
                                nc.vector.tensor_tensor(
                                    hsb[:, csl], gates[2][:, csl],
                                    tct[:, csl], Alu.mult,
                                )
                                nc.sync.dma_start(
                                    out=h_out.ap()[bsl, csl], in_=hsb[:, csl]
                                )
                        else:
                            nc.gpsimd.tensor_tensor(
                                hsb[:], gates[2][:], tct[:], Alu.mult
                            )
                            nc.sync.dma_start(out=h_out.ap()[bsl, :], in_=hsb[:])

    nc.compile()
    _cached[key] = nc
    return nc


def _prep(X_t, c_prev, W_f, W_i, W_o, W_c, U_f, U_i, U_o, U_c, b_f, b_i, b_o, b_c):
    """Host-side (free) preprocessing: concat, cast, transpose, shard."""
    f16 = np.float16
    W = np.ascontiguousarray(
        np.concatenate([W_f, W_i, W_o, W_c], axis=1).astype(f16)
    )
    U = np.ascontiguousarray(
        np.concatenate([U_f, U_i, U_o, U_c], axis=1).astype(f16)
    )
    b = np.concatenate([b_f, b_i, b_o, b_c], axis=0).astype(np.float32)
    has_bias = bool(np.any(b != 0.0))

    X16 = np.asarray(X_t).astype(f16)
    C16 = np.asarray(c_prev).astype(f16)
    C32 = np.asarray(c_prev).astype(np.float32)

    in_maps = []
    for i in range(N_CORES):
        sl = slice(i * BL, (i + 1) * BL)
        m = {
            "xT_f16": np.ascontiguousarray(X16[sl].T),
            "cT_f16": np.ascontiguousarray(C16[sl].T),
            "c_f32": np.ascontiguousarray(C32[sl]),
            "w_f16": W,
            "u_f16": U,
        }
        if has_bias:
            m["bias_bc"] = np.ascontiguousarray(
                np.broadcast_to(b[None, :], (128, G4))
            )
        in_maps.append(m)
    return in_maps, has_bias


def kernel(**inputs):
    in_maps, has_bias = _prep(**inputs)
    nc = _build(has_bias)
    last_err = None
    for _ in range(3):
        try:
            res = bass_utils.run_bass_kernel_spmd(
                nc, in_maps, core_ids=list(range(N_CORES))
            )
            break
        except Exception as e:  # intermittent device wedge: retry
            last_err = e
            import time
            time.sleep(5)
    else:
        raise last_err
    return np.concatenate([res.results[i]["h_out"] for i in range(N_CORES)], axis=0)



# revision 2
# speedup vs baseline: 1.1395x; 1.1395x over previous
"""Peephole-LSTM cell fused kernel for 8 Trainium2 NeuronCores.

Math (per reference):
    pre = X_t @ W + c_prev @ U + b          # W/U/b are the 4 gates concat'd
    f, i, o, c_hat = split(pre);  f,i,o = sigmoid;  c_hat = tanh
    c_t = f * c_prev + i * c_hat
    h_t = o * tanh(c_t)

Sharding: data-parallel over the batch dim (16384 -> 8 x 2048), weights
replicated, no cross-device communication.

Per-core device plan (B_loc=2048, D=512, 4H=2048):
  - Host pre-transposes X and c_prev and casts to fp16 (free), so the
    device does nothing but 512 [128k x 128m x 512n] fp16 matmuls plus
    the elementwise gate chain.
  - CRITICAL scheduling rule: PSUM accumulation groups are NEVER
    interleaved.  A k-outer pattern that round-robins matmuls across
    several open PSUM banks drops the PE clock from ~2.37 GHz to
    ~1.98 GHz for the REST OF THE KERNEL (259 ns vs 215.8 ns per
    matmul, +23 us total).  Every chain here runs start..stop to
    completion before the next group opens.
  - Startup: operands stream via 4 serial DMA lanes in consumption
    order (X^T/W first, C^T/U, then late batch-halves).  While the
    first pieces land (HBM-bound), junk matmuls ramp the PE clock.
  - Phase A: X@W only for the first N_STAGED tiles as sequential
    4-chains, staged to SBUF fp16 (PX); phase B adds C@U and the
    staged X@W back.  Staged and fused tiles are interleaved in phase
    B so the extra vector adds of staged tiles spread across a wider
    window.
  - Elementwise: scalar does sigmoid/tanh straight from PSUM; the
    c_t/h chain is split across VectorE and GpSimd; c_prev is
    re-loaded fp32-natural for full precision in f * c_prev.  The
    final tile runs a chunked all-fast-engine tail so the exposed
    time after the last matmul is ~1.5 us.
"""

import sys

if "/opt/trn_rl_repo" not in sys.path:
    sys.path.insert(0, "/opt/trn_rl_repo")

import numpy as np

import concourse.bacc as bacc
import concourse.mybir as mybir
import concourse.tile as tile
from concourse import bass_utils

N_CORES = 8
B, D, H = 16384, 512, 512
BL = B // N_CORES          # 2048 rows per core
G4 = 4 * H                 # 2048, the concatenated gate dim
KT = D // 128              # 4 k-tiles
BT = BL // 128             # 16 batch tiles per core
WARMUP_MMS = 14            # junk matmuls to lift the HAM clock gate
N_STAGED = 4               # batch tiles that run X@W-only while C/U load
BANK_ORDER = (3, 1, 0, 2)  # c_hat, i, f, o — deep chain first
LAST_ORDER = (3, 0, 1, 2)  # final tile: f early so c_t closes pre-end
# staged tiles interleaved with fused ones to spread their vector adds
TILE_ORDER = (0, 4, 1, 5, 2, 6, 3, 7, 8, 9, 10, 11, 12, 13, 14, 15)

_cached = {}


def _build(has_bias: bool):
    key = has_bias
    if key in _cached:
        return _cached[key]

    f32 = mybir.dt.float32
    f16 = mybir.dt.float16
    AF = mybir.ActivationFunctionType
    Alu = mybir.AluOpType

    nc = bacc.Bacc("TRN2", target_bir_lowering=False, debug=False,
                   enable_asserts=False, enable_partition_id=False)
    xT = nc.dram_tensor("xT_f16", [D, BL], f16, kind="ExternalInput")
    cT = nc.dram_tensor("cT_f16", [D, BL], f16, kind="ExternalInput")
    c_f32 = nc.dram_tensor("c_f32", [BL, D], f32, kind="ExternalInput")
    w_f16 = nc.dram_tensor("w_f16", [D, G4], f16, kind="ExternalInput")
    u_f16 = nc.dram_tensor("u_f16", [D, G4], f16, kind="ExternalInput")
    if has_bias:
        bias_bc = nc.dram_tensor("bias_bc", [128, G4], f32, kind="ExternalInput")
    h_out = nc.dram_tensor("h_out", [BL, H], f32, kind="ExternalOutput")

    with tile.TileContext(nc) as tc:
        with (
            tc.tile_pool(name="const", bufs=1) as const,
            tc.tile_pool(name="px", bufs=1) as px_p,
            tc.tile_pool(name="psum", bufs=8, space="PSUM") as psum,
            tc.tile_pool(name="cnat", bufs=4) as cnat,
            tc.tile_pool(name="gates", bufs=8) as gate_p,
            tc.tile_pool(name="tmp1", bufs=4) as tmp1_p,
            tc.tile_pool(name="tmp2", bufs=4) as tmp2_p,
            tc.tile_pool(name="hsb", bufs=3) as h_p,
        ):
            # PE warm-up: burn the boot->operand-arrival window on junk
            # matmuls so the HAM clock gate sees sustained activity and
            # the real stream starts at full clock.  gpsimd memset (its
            # sequencer preamble finishes earliest) so the first junk
            # matmul issues ~2 us sooner than with a vector memset.
            junk = const.tile([128, 512], f16, tag="junk", name="junk")
            nc.gpsimd.memset(junk[:], 0.0)
            # Pre-create the phase-A PSUM tiles for the first two staged
            # tiles and aim the junk matmuls at them: each real chain
            # begins start=True (resets the bank), and targeting
            # later-read tiles keeps bacc's DCE from dropping the junk.
            psA01 = {
                (bt, jc): psum.tile([128, 512], f32, tag="ps",
                                    name=f"psA{bt}_{jc}")
                for bt in (0, 1) for jc in range(4)
            }
            for i in range(WARMUP_MMS):
                tgt = psA01[((i // 4) % 2, i % 4)]
                nc.tensor.matmul(
                    tgt[:], junk[:, 0:128], junk[:], start=True, stop=True
                )

            # 32 operand pieces of [128, 1024] fp16 (256 KiB each):
            # xt/ct split by batch half (a = batch tiles 0-7), w/u by
            # gate-pair half (b-half feeds gates 3,2; a-half gates 1,0).
            def piece(name):
                return const.tile([128, 1024], f16, tag=name, name=name)

            XTh = [[piece(f"xt{k}h{q}") for q in range(2)] for k in range(KT)]
            CTh = [[piece(f"ct{k}h{q}") for q in range(2)] for k in range(KT)]
            Wh = [[piece(f"w{k}h{j}") for j in range(2)] for k in range(KT)]
            Uh = [[piece(f"u{k}h{j}") for j in range(2)] for k in range(KT)]

            def dram(t, k, half):
                return t.ap()[k * 128:(k + 1) * 128,
                              half * 1024:(half + 1) * 1024]

            # Priority order = consumption order.  Phase A runs gates
            # (3,2,1,0) per tile: W b-half pieces are needed first, the
            # a-half four chains later.  Same for U in phase B's staged
            # chains (BANK_ORDER starts at 3).
            prio = []
            for k in range(KT):
                prio.append((XTh[k][0], xT, k, 0))
                prio.append((Wh[k][1], w_f16, k, 1))
            for k in range(KT):
                prio.append((Wh[k][0], w_f16, k, 0))
            for k in range(KT):
                prio.append((CTh[k][0], cT, k, 0))
                prio.append((Uh[k][1], u_f16, k, 1))
            for k in range(KT):
                prio.append((Uh[k][0], u_f16, k, 0))
            block2_end = prio[-1][0]
            for k in range(KT):
                prio.append((XTh[k][1], xT, k, 1))
            for k in range(KT):
                prio.append((CTh[k][1], cT, k, 1))

            # 4 lanes (strided), issued on sync/scalar (a gated DMA
            # blocks its issuing engine).  Each piece gates on the piece
            # TWO back in its lane: 2 transfers in flight per lane hide
            # per-piece semaphore latency while keeping arrival order.
            n_lanes = 4
            lanes = [prio[i::n_lanes] for i in range(n_lanes)]
            for wave in range(len(lanes[0])):
                for li, lane in enumerate(lanes):
                    if wave >= len(lane):
                        continue
                    dst, t, k, half = lane[wave]
                    if wave > 1:
                        prev = lane[wave - 2][0]
                        nc.vector.tensor_copy(dst[0:1, 0:1], prev[0:1, 0:1])
                        eng = nc.sync
                    else:
                        eng = nc.scalar if li % 2 else nc.sync
                    eng.dma_start(out=dst[:], in_=dram(t, k, half))
            if has_bias:
                bias_sb = const.tile([128, G4], f32, tag="bias")
                nc.sync.dma_start(out=bias_sb[:], in_=bias_bc.ap())

            def lhsq(T, k, bt):
                q, r = divmod(bt, 8)
                return T[k][q][:, r * 128:(r + 1) * 128]

            def rhsj(T, k, jc):
                q, r = divmod(jc, 2)
                return T[k][q][:, r * 512:(r + 1) * 512]

            # Phase A: X@W only for staged tiles, SEQUENTIAL 4-chains
            # (one PSUM group at a time), evicted to SBUF fp16.
            PX = []
            for bt in range(N_STAGED):
                PX.append(px_p.tile([128, G4], f16, tag=f"px{bt}",
                                    name=f"px{bt}"))
            for bt in range(N_STAGED):
                for jc in (3, 2, 1, 0):
                    if (bt, jc) in psA01:
                        ps = psA01[(bt, jc)]
                    else:
                        ps = psum.tile([128, 512], f32, tag="ps",
                                       name=f"psA{bt}_{jc}")
                    for k in range(KT):
                        nc.tensor.matmul(
                            ps[:], lhsq(XTh, k, bt), rhsj(Wh, k, jc),
                            start=(k == 0), stop=(k == KT - 1),
                        )
                    nc.scalar.activation(
                        PX[bt][:, jc * 512:(jc + 1) * 512], ps[:], AF.Copy
                    )

            first_cn = True
            # Phase B: per tile, fill the four gate banks (sequential
            # chains) and run the eviction/elementwise chain.
            for bt in TILE_ORDER:
                bsl = slice(bt * 128, (bt + 1) * 128)
                last = bt == BT - 1
                cn = cnat.tile([128, H], f32, tag="cn", name=f"cn{bt}")
                if first_cn:
                    # Keep c_prev-natural loads behind the operand lanes.
                    nc.vector.tensor_copy(cn[0:1, 0:1], block2_end[0:1, 0:1])
                    first_cn = False
                nc.sync.dma_start(out=cn[:], in_=c_f32.ap()[bsl, :])

                staged = bt < N_STAGED
                order = LAST_ORDER if last else BANK_ORDER
                gates = {}
                for jc in order:
                    jsl = slice(jc * 512, (jc + 1) * 512)
                    ps = psum.tile([128, 512], f32, tag="ps",
                                   name=f"psB{bt}_{jc}")
                    if staged:
                        for k in range(KT):
                            nc.tensor.matmul(
                                ps[:], lhsq(CTh, k, bt), rhsj(Uh, k, jc),
                                start=(k == 0), stop=(k == KT - 1),
                            )
                        nc.vector.tensor_tensor(
                            ps[:], ps[:], PX[bt][:, jsl], Alu.add
                        )
                    else:
                        for k in range(KT):
                            nc.tensor.matmul(
                                ps[:], lhsq(XTh, k, bt), rhsj(Wh, k, jc),
                                start=(k == 0), stop=False,
                            )
                        for k in range(KT):
                            nc.tensor.matmul(
                                ps[:], lhsq(CTh, k, bt), rhsj(Uh, k, jc),
                                start=False, stop=(k == KT - 1),
                            )
                    if has_bias:
                        nc.vector.tensor_tensor(
                            ps[:], ps[:], bias_sb[:, jsl], Alu.add
                        )
                    g = gate_p.tile([128, 512], f32, tag="g",
                                    name=f"g{bt}_{jc}")
                    if last and jc == 2:
                        # chunked: evict/multiply/store the o-gate in
                        # 128-col pieces so the exposed tail shrinks.
                        for hs in range(4):
                            csl = slice(hs * 128, (hs + 1) * 128)
                            nc.scalar.activation(
                                g[:, csl], ps[:, csl], AF.Sigmoid
                            )
                    else:
                        nc.scalar.activation(
                            g[:], ps[:], AF.Tanh if jc == 3 else AF.Sigmoid
                        )
                    gates[jc] = g
                    # Chain steps as their inputs become ready.
                    if jc == 0:
                        t1 = tmp1_p.tile([128, H], f32, tag="t1",
                                         name=f"t1_{bt}")
                        nc.gpsimd.tensor_tensor(
                            t1[:], gates[0][:], cn[:], Alu.mult
                        )
                    elif jc == 1:
                        t2 = tmp2_p.tile([128, H], f32, tag="t2",
                                         name=f"t2_{bt}")
                        nc.vector.tensor_tensor(
                            t2[:], gates[1][:], gates[3][:], Alu.mult
                        )
                        if last:
                            nc.vector.tensor_tensor(t1[:], t1[:], t2[:],
                                                    Alu.add)
                            tct = tmp2_p.tile([128, H], f32, tag="tct",
                                              name=f"tct{bt}")
                            for hs in range(2):
                                csl = slice(hs * 256, (hs + 1) * 256)
                                nc.scalar.activation(tct[:, csl], t1[:, csl],
                                                     AF.Tanh)
                    elif jc == 2:
                        if not last:
                            nc.vector.tensor_tensor(t1[:], t1[:], t2[:],
                                                    Alu.add)
                            tct = tmp2_p.tile([128, H], f32, tag="tct",
                                              name=f"tct{bt}")
                            nc.scalar.activation(tct[:], t1[:], AF.Tanh)
                        hsb = h_p.tile([128, H], f32, tag="h", name=f"h{bt}")
                        if last:
                            for hs in range(4):
                                csl = slice(hs * 128, (hs + 1) * 128)
                                nc.vector.tensor_tensor(
                                    hsb[:, csl], gates[2][:, csl],
                                    tct[:, csl], Alu.mult,
                                )
                                nc.sync.dma_start(
                                    out=h_out.ap()[bsl, csl], in_=hsb[:, csl]
                                )
                        else:
                            nc.gpsimd.tensor_tensor(
                                hsb[:], gates[2][:], tct[:], Alu.mult
                            )
                            nc.sync.dma_start(out=h_out.ap()[bsl, :],
                                              in_=hsb[:])

    nc.compile()
    _cached[key] = nc
    return nc


def _prep(X_t, c_prev, W_f, W_i, W_o, W_c, U_f, U_i, U_o, U_c, b_f, b_i, b_o, b_c):
    """Host-side (free) preprocessing: concat, cast, transpose, shard."""
    f16 = np.float16
    W = np.ascontiguousarray(
        np.concatenate([W_f, W_i, W_o, W_c], axis=1).astype(f16)
    )
    U = np.ascontiguousarray(
        np.concatenate([U_f, U_i, U_o, U_c], axis=1).astype(f16)
    )
    b = np.concatenate([b_f, b_i, b_o, b_c], axis=0).astype(np.float32)
    has_bias = bool(np.any(b != 0.0))

    X16 = np.asarray(X_t).astype(f16)
    C16 = np.asarray(c_prev).astype(f16)
    C32 = np.asarray(c_prev).astype(np.float32)

    in_maps = []
    for i in range(N_CORES):
        sl = slice(i * BL, (i + 1) * BL)
        m = {
            "xT_f16": np.ascontiguousarray(X16[sl].T),
            "cT_f16": np.ascontiguousarray(C16[sl].T),
            "c_f32": np.ascontiguousarray(C32[sl]),
            "w_f16": W,
            "u_f16": U,
        }
        if has_bias:
            m["bias_bc"] = np.ascontiguousarray(
                np.broadcast_to(b[None, :], (128, G4))
            )
        in_maps.append(m)
    return in_maps, has_bias


def kernel(**inputs):
    in_maps, has_bias = _prep(**inputs)
    nc = _build(has_bias)
    last_err = None
    for _ in range(3):
        try:
            res = bass_utils.run_bass_kernel_spmd(
                nc, in_maps, core_ids=list(range(N_CORES))
            )
            break
        except Exception as e:  # intermittent device wedge: retry
            last_err = e
            import time
            time.sleep(5)
    else:
        raise last_err
    return np.concatenate([res.results[i]["h_out"] for i in range(N_CORES)],
                          axis=0)
